# revision 21
# baseline (speedup 1.0000x reference)
"""Grouped-linear (EvolvedLoopLinear) Trainium2 Bass kernel.

Problem: out[b, j] = sum_s x[b, g*64+s] * weight[j, g*64+s] + bias[j],
with g = j % 128, for x [4096, 8192], weight [4096, 8192], bias [4096].

Only a gathered [4096, 64] slice of the weight matrix is live, so the
kernel is pure memory streaming: read x, write out.  Strategy:

  - Data-parallel over batch across 8 cores (512 rows each).
  - All layout work happens on the host: x is packed per-core into a
    PE-ready transposed fp16 layout xt[64h+s, 512k+b] = x[b, 64(2k+h)+s]
    (pair k = groups 2k,2k+1 stacked on the 128 partitions), the live
    weight slice into block-diagonal pair stationaries, and the bias into
    the transposed-output partition layout.  fp16 halves HBM traffic
    (measured end-to-end quantization error ~4.6e-4 vs the f32 oracle).
  - Per core the device program is a straight stream: 8x 1MiB x-slab
    loads (sync queue), 64 pair matmuls (fp16, f32 PSUM, two pairs
    stacked per PSUM bank on partition halves), ACT/DVE-alternating
    bias-add evacuations casting to fp16, 4x 1MiB output stores (scalar
    queue).  No on-device transposes; the PE only does the 2*64*512
    columns of real matmul work.
  - The transposed fp16 output [128, 512*32] is unscrambled to
    [4096, 4096] f32 on the host.
"""
import os as _os
import numpy as np
from contextlib import ExitStack

import concourse.bass as bass
import concourse.tile as tile
import concourse.tile_sem_assignment as _tsa
from concourse import bacc, mybir
from concourse.bass_utils import run_bass_kernel_spmd

# The walrus build in this container rejects instructions carrying more than
# a couple of semaphore waits ("Too many sync wait commands"); capping the
# HWDGE completion lanes keeps the kernel-tail drain under that limit.
_tsa.NUM_HWDGE_SEMS = int(_os.environ.get("K_HWSEMS", "8"))

if _os.environ.get("K_LDWOPT", "0") == "1":
    # let walrus use the PE background weight buffer (overlaps LDWEIGHTS
    # with in-flight matmuls; safe here: the PE stream is matmul-only)
    import concourse.bass_utils as _bu
    if not getattr(_bu, "_ldwopt_patched", False):
        _orig_run_command = _bu.run_command

        def _patched_run_command(argv, **kwargs):
            argv = ["--enable-ldw-opt=true" if a == "--enable-ldw-opt=false"
                    else a for a in argv]
            return _orig_run_command(argv, **kwargs)

        _bu.run_command = _patched_run_command
        _bu._ldwopt_patched = True

BATCH = 4096
IN_F = 8192
OUT_F = 4096
GROUPS = 128
STEP = 64
M_PER_G = 32          # outputs per group
N_CORES = 8
B_CORE = BATCH // N_CORES      # 512
N_PAIR = GROUPS // 2           # 64 group pairs (k: groups 2k, 2k+1)
N_SUPER = N_PAIR // 2          # 32 super-pairs (t: pairs 2t, 2t+1)


def _intlist(env, default):
    v = _os.environ.get(env)
    return [int(s) for s in v.split(",")] if v else default


# x slab sizes in pairs (128 KiB/pair); finer at the tail so the last
# matmuls aren't gated on a full 1 MiB transfer
SLAB_PAIRS = _intlist("K_SLABS", [12, 12, 12, 12, 8, 4, 2, 1, 1])
assert sum(SLAB_PAIRS) == N_PAIR
# out tile sizes in super-pairs (128 KiB/super); finer at the tail
OTILE_SUPERS = _intlist("K_OTILES", [8, 8, 8, 4, 2, 1, 1])
assert sum(OTILE_SUPERS) == N_SUPER
# which slab loads ride the scalar (ACT) HWDGE ring: ~1/3 of the read
# bytes, so during the store-overlap phase the packet round-robin between
# the two rings gives reads close to their 2:1 byte share
SCALAR_SLABS = set(_intlist("K_SCALAR_SLABS", [1, 4]))
EVAC = _os.environ.get("K_EVAC", "mix")   # dve | mix
# otile stores riding the sync ring (empty at the tail, so the final small
# stores aren't queued behind earlier 1 MiB stores on the ACT ring)
SYNC_OTILES = set(_intlist("K_SYNC_OTILES", [3, 4, 5, 6]))

f32 = mybir.dt.float32
f16 = mybir.dt.float16

WARMUP_MM = int(_os.environ.get("K_WARMUP", "4"))
TILEPOS = _os.environ.get("K_TILEPOS", "1") == "1"

_COMPILED = {}


def _build():
    if "nc" in _COMPILED:
        return _COMPILED["nc"]

    nc = bacc.Bacc("TRN2", target_bir_lowering=False, debug=False)
    x_ap = nc.dram_tensor("x_s", [128, N_PAIR * B_CORE], f16,
                          kind="ExternalInput").ap()
    # compact params: wc[64h+s, 32k+m] = Wg[m*128+2k+h, s] (cols 0:2048),
    # bias_p fp16 (cols 2048:2048+32) -- one 0.53 MiB DMA, no zero padding
    w_ap = nc.dram_tensor("w_c", [128, N_PAIR * M_PER_G + N_SUPER], f16,
                          kind="ExternalInput").ap()
    y_ap = nc.dram_tensor("out_s", [128, N_SUPER * B_CORE], f16,
                          kind="ExternalOutput").ap()

    with tile.TileContext(nc) as tc:
        with ExitStack() as ctx:
            const_pool = ctx.enter_context(tc.tile_pool(name="const", bufs=1))
            slab_pool = ctx.enter_context(
                tc.tile_pool(name="slab", bufs=len(SLAB_PAIRS)))
            osb_pool = ctx.enter_context(
                tc.tile_pool(name="osb", bufs=len(OTILE_SUPERS)))
            ps_pool = ctx.enter_context(tc.tile_pool(name="ps", bufs=6,
                                                     space="PSUM"))

            # params FIRST on the sync queue (ahead of the slabs) so they
            # land before slab0 and never ride the congested stream; stores
            # go down the ACT HWDGE queue so the read FIFO stays pure
            NWC = N_PAIR * M_PER_G
            wc_sb = const_pool.tile([128, NWC + N_SUPER], f16)
            nc.sync.dma_start(wc_sb[:], w_ap[:])

            # expand to the block-diagonal pair stationaries on-device:
            # w_sb[64h+s, 64k+32h'+m] = wc[64h+s, 32k+m] if h==h' else 0
            w_sb = const_pool.tile([128, N_PAIR * 64], f16)
            nc.gpsimd.memset(w_sb[:], 0)
            for h in range(2):
                dst = w_sb[64 * h:64 * h + 64, :].rearrange(
                    "p (k u) -> p k u", u=64)[:, :, 32 * h:32 * h + 32]
                srcv = wc_sb[64 * h:64 * h + 64, 0:NWC].rearrange(
                    "p (k m) -> p k m", m=M_PER_G)
                nc.vector.tensor_copy(dst, srcv)
            bias_sb = const_pool.tile([128, N_SUPER], f32)
            nc.vector.tensor_copy(bias_sb[:], wc_sb[:, NWC:NWC + N_SUPER])

            # slab_of_pair[k] -> (tile, col offset)
            slab_of_pair = {}
            p0 = 0
            for si, np_ in enumerate(SLAB_PAIRS):
                s = slab_pool.tile([128, np_ * B_CORE], f16, tag="slab",
                                   name=f"slab{si}")
                eng = nc.scalar if si in SCALAR_SLABS else nc.sync
                eng.dma_start(
                    s[:], x_ap[:, p0 * B_CORE:(p0 + np_) * B_CORE])
                for j in range(np_):
                    slab_of_pair[p0 + j] = (s, j * B_CORE)
                p0 += np_

            if WARMUP_MM:
                # pull the PE activity monitor to full clock before the
                # first real matmuls issue
                wm = ps_pool.tile([64, 64], f32, tag="ps", name="warm")
                for _ in range(WARMUP_MM):
                    nc.tensor.matmul(wm[:], w_sb[:, 0:64], wc_sb[:, 0:64],
                                     start=True, stop=True)

            t0 = 0
            for C, ns_ in enumerate(OTILE_SUPERS):
                ot = osb_pool.tile([128, ns_ * B_CORE], f16, tag="osb",
                                   name=f"osb{C}")
                for t2 in range(ns_):
                    t = t0 + t2
                    ps = ps_pool.tile([128, B_CORE], f32, tag="ps")
                    for u in range(2):
                        k = 2 * t + u
                        slab, off = slab_of_pair[k]
                        # column-tiling: the two pair matmuls land on
                        # disjoint PE column groups and stream concurrently
                        kw = dict(tile_position=(0, 64 * u)) if TILEPOS else {}
                        nc.tensor.matmul(
                            ps[64 * u:64 * u + 64, :],
                            w_sb[:, k * 64:(k + 1) * 64],
                            slab[:, off:off + B_CORE],
                            start=True, stop=True, **kw)
                    # bias-add evacuation, fp16 cast on write.  all-DVE
                    # keeps InstActivation out of the program (no ACT table
                    # load in the preamble) and the scalar engine free to
                    # issue stores promptly
                    dst = ot[:, t2 * B_CORE:(t2 + 1) * B_CORE]
                    if EVAC == "mix" and t2 % 2 == 0:
                        nc.scalar.add(dst, ps[:], bias_sb[:, t:t + 1])
                    else:
                        nc.vector.tensor_scalar_add(dst, ps[:],
                                                    bias_sb[:, t:t + 1])
                oeng = nc.sync if C in SYNC_OTILES else nc.scalar
                oeng.dma_start(
                    y_ap[:, t0 * B_CORE:(t0 + ns_) * B_CORE], ot[:])
                t0 += ns_

    nc.compile()
    _COMPILED["nc"] = nc
    return nc


def _prep_in_maps(x, weight, bias):
    x = np.asarray(x, dtype=np.float32)
    weight = np.asarray(weight, dtype=np.float32)
    bias = np.asarray(bias, dtype=np.float32)

    # x -> per-core PE-ready transposed fp16: xt[c][64h+s, 512k+b]
    #    = x[512c+b, 64(2k+h)+s]
    xt = x.reshape(N_CORES, B_CORE, N_PAIR, 2, STEP)    # [c, b, k, h, s]
    xt = xt.transpose(0, 3, 4, 2, 1)                    # [c, h, s, k, b]
    xt = np.ascontiguousarray(xt, dtype=np.float16)
    xt = xt.reshape(N_CORES, 128, N_PAIR * B_CORE)

    # gathered weight slice: Wg[j, s] = weight[j, (j%128)*64 + s]
    j = np.arange(OUT_F)
    Wg = weight.reshape(OUT_F, GROUPS, STEP)[j, j % GROUPS]   # [4096, 64]
    Wk = Wg.reshape(M_PER_G, N_PAIR, 2, STEP)                 # [m, k, h, s]
    # compact stationary halves: wc[64h+s, 32k+m] = Wk[m, k, h, s]
    wc = np.empty((2, STEP, N_PAIR, M_PER_G), dtype=np.float32)
    for h in range(2):
        wc[h] = Wk[:, :, h, :].transpose(2, 1, 0)             # [s, k, m]
    wc = wc.reshape(128, N_PAIR * M_PER_G)

    # bias in transposed-output layout: bias_p[64u+32h+m, t]
    #    = bias[m*128 + 4t + 2u + h]
    b4 = bias.reshape(M_PER_G, N_SUPER, 2, 2)                 # [m, t, u, h]
    bias_p = b4.transpose(2, 3, 0, 1).reshape(128, N_SUPER)

    w_c = np.ascontiguousarray(
        np.concatenate([wc, bias_p], axis=1), dtype=np.float16)

    in_maps = []
    for c in range(N_CORES):
        in_maps.append({
            "x_s": xt[c],
            "w_c": w_c,
        })
    return in_maps


def _unscramble(results):
    # y[64u+32h+m, 512t+b] = out[512c+b, m*128 + 4t + 2u + h]
    out = np.empty((BATCH, OUT_F), dtype=np.float32)
    for c in range(N_CORES):
        y = np.asarray(results[c]["out_s"])                  # [128, 16384] f16
        o = y.reshape(2, 2, M_PER_G, N_SUPER, B_CORE)        # [u, h, m, t, b]
        o = o.transpose(4, 2, 3, 0, 1)                       # [b, m, t, u, h]
        out[c * B_CORE:(c + 1) * B_CORE] = o.reshape(
            B_CORE, OUT_F).astype(np.float32)
    return out


def kernel(x, weight, bias):
    nc = _build()
    in_maps = _prep_in_maps(x, weight, bias)
    res = run_bass_kernel_spmd(nc, in_maps, core_ids=list(range(N_CORES)))
    return _unscramble(res.results)


# revision 22
# speedup vs baseline: 1.1206x; 1.1206x over previous
"""Grouped-linear (EvolvedLoopLinear) Trainium2 Bass kernel.

Problem: out[b, j] = sum_s x[b, g*64+s] * weight[j, g*64+s] + bias[j],
with g = j % 128, for x [4096, 8192], weight [4096, 8192], bias [4096].

Only a gathered [4096, 64] slice of the weight matrix is live, so the
kernel is pure memory streaming: read x, write out.  Strategy:

  - Data-parallel over batch across 8 cores (512 rows each).
  - All layout work happens on the host: x is packed per-core into a
    PE-ready transposed fp16 layout xt[64h+s, 512k+b] = x[b, 64(2k+h)+s]
    (pair k = groups 2k,2k+1 stacked on the 128 partitions), the live
    weight slice into block-diagonal pair stationaries, and the bias into
    the transposed-output partition layout.  fp16 halves HBM traffic
    (measured end-to-end quantization error ~4.6e-4 vs the f32 oracle).
  - Per core the device program is a straight stream: 8x 1MiB x-slab
    loads (sync queue), 64 pair matmuls (fp16, f32 PSUM, two pairs
    stacked per PSUM bank on partition halves), ACT/DVE-alternating
    bias-add evacuations casting to fp16, 4x 1MiB output stores (scalar
    queue).  No on-device transposes; the PE only does the 2*64*512
    columns of real matmul work.
  - The transposed fp16 output [128, 512*32] is unscrambled to
    [4096, 4096] f32 on the host.
"""
import os as _os
import numpy as np
from contextlib import ExitStack

import concourse.bass as bass
import concourse.tile as tile
import concourse.tile_sem_assignment as _tsa
from concourse import bacc, mybir
from concourse.bass_utils import run_bass_kernel_spmd

# The walrus build in this container rejects instructions carrying more than
# a couple of semaphore waits ("Too many sync wait commands"); capping the
# HWDGE completion lanes keeps the kernel-tail drain under that limit.
_tsa.NUM_HWDGE_SEMS = int(_os.environ.get("K_HWSEMS", "8"))

if _os.environ.get("K_LDWOPT", "0") == "1":
    # let walrus use the PE background weight buffer (overlaps LDWEIGHTS
    # with in-flight matmuls; safe here: the PE stream is matmul-only)
    import concourse.bass_utils as _bu
    if not getattr(_bu, "_ldwopt_patched", False):
        _orig_run_command = _bu.run_command

        def _patched_run_command(argv, **kwargs):
            argv = ["--enable-ldw-opt=true" if a == "--enable-ldw-opt=false"
                    else a for a in argv]
            return _orig_run_command(argv, **kwargs)

        _bu.run_command = _patched_run_command
        _bu._ldwopt_patched = True

BATCH = 4096
IN_F = 8192
OUT_F = 4096
GROUPS = 128
STEP = 64
M_PER_G = 32          # outputs per group
N_CORES = 8
B_CORE = BATCH // N_CORES      # 512
N_PAIR = GROUPS // 2           # 64 group pairs (k: groups 2k, 2k+1)
N_SUPER = N_PAIR // 2          # 32 super-pairs (t: pairs 2t, 2t+1)


def _intlist(env, default):
    v = _os.environ.get(env)
    return [int(s) for s in v.split(",")] if v else default


# x slab sizes in pairs (128 KiB/pair); finer at the tail so the last
# matmuls aren't gated on a full 1 MiB transfer
SLAB_PAIRS = _intlist("K_SLABS", [12, 12, 12, 12, 8, 4, 2, 1, 1])
assert sum(SLAB_PAIRS) == N_PAIR
# out tile sizes in super-pairs (128 KiB/super); finer at the tail
OTILE_SUPERS = _intlist("K_OTILES", [8, 8, 8, 4, 2, 1, 1])
assert sum(OTILE_SUPERS) == N_SUPER
# which slab loads ride the scalar (ACT) HWDGE ring: ~1/3 of the read
# bytes, so during the store-overlap phase the packet round-robin between
# the two rings gives reads close to their 2:1 byte share
SCALAR_SLABS = set(_intlist("K_SCALAR_SLABS", [1, 4]))
EVAC = _os.environ.get("K_EVAC", "mix")   # dve | mix
# otile stores riding the sync ring (empty at the tail, so the final small
# stores aren't queued behind earlier 1 MiB stores on the ACT ring)
SYNC_OTILES = set(_intlist("K_SYNC_OTILES", [-1]))

f32 = mybir.dt.float32
f16 = mybir.dt.float16

WARMUP_MM = int(_os.environ.get("K_WARMUP", "4"))
TILEPOS = _os.environ.get("K_TILEPOS", "1") == "1"

_COMPILED = {}


def _build():
    if "nc" in _COMPILED:
        return _COMPILED["nc"]

    nc = bacc.Bacc("TRN2", target_bir_lowering=False, debug=False)
    x_ap = nc.dram_tensor("x_s", [128, N_PAIR * B_CORE], f16,
                          kind="ExternalInput").ap()
    # compact params: wc[64h+s, 32k+m] = Wg[m*128+2k+h, s] (cols 0:2048),
    # bias_p fp16 (cols 2048:2048+32) -- one 0.53 MiB DMA, no zero padding
    w_ap = nc.dram_tensor("w_c", [128, N_PAIR * M_PER_G + N_SUPER], f16,
                          kind="ExternalInput").ap()
    y_ap = nc.dram_tensor("out_s", [128, N_SUPER * B_CORE], f16,
                          kind="ExternalOutput").ap()

    with tile.TileContext(nc) as tc:
        with ExitStack() as ctx:
            const_pool = ctx.enter_context(tc.tile_pool(name="const", bufs=1))
            slab_pool = ctx.enter_context(
                tc.tile_pool(name="slab", bufs=len(SLAB_PAIRS)))
            osb_pool = ctx.enter_context(
                tc.tile_pool(name="osb", bufs=len(OTILE_SUPERS)))
            ps_pool = ctx.enter_context(tc.tile_pool(name="ps", bufs=6,
                                                     space="PSUM"))

            # params FIRST on the sync queue (ahead of the slabs) so they
            # land before slab0 and never ride the congested stream; stores
            # go down the ACT HWDGE queue so the read FIFO stays pure
            NWC = N_PAIR * M_PER_G
            wc_sb = const_pool.tile([128, NWC + N_SUPER], f16)
            nc.sync.dma_start(wc_sb[:], w_ap[:])

            # expand to the block-diagonal pair stationaries on-device:
            # w_sb[64h+s, 64k+32h'+m] = wc[64h+s, 32k+m] if h==h' else 0
            w_sb = const_pool.tile([128, N_PAIR * 64], f16)
            nc.gpsimd.memset(w_sb[:], 0)
            for h in range(2):
                dst = w_sb[64 * h:64 * h + 64, :].rearrange(
                    "p (k u) -> p k u", u=64)[:, :, 32 * h:32 * h + 32]
                srcv = wc_sb[64 * h:64 * h + 64, 0:NWC].rearrange(
                    "p (k m) -> p k m", m=M_PER_G)
                nc.vector.tensor_copy(dst, srcv)
            bias_sb = const_pool.tile([128, N_SUPER], f32)
            nc.vector.tensor_copy(bias_sb[:], wc_sb[:, NWC:NWC + N_SUPER])

            # slab_of_pair[k] -> (tile, col offset)
            slab_of_pair = {}
            p0 = 0
            for si, np_ in enumerate(SLAB_PAIRS):
                s = slab_pool.tile([128, np_ * B_CORE], f16, tag="slab",
                                   name=f"slab{si}")
                eng = nc.scalar if si in SCALAR_SLABS else nc.sync
                eng.dma_start(
                    s[:], x_ap[:, p0 * B_CORE:(p0 + np_) * B_CORE])
                for j in range(np_):
                    slab_of_pair[p0 + j] = (s, j * B_CORE)
                p0 += np_

            if WARMUP_MM:
                # pull the PE activity monitor to full clock before the
                # first real matmuls issue
                wm = ps_pool.tile([64, 64], f32, tag="ps", name="warm")
                for _ in range(WARMUP_MM):
                    nc.tensor.matmul(wm[:], w_sb[:, 0:64], wc_sb[:, 0:64],
                                     start=True, stop=True)

            t0 = 0
            for C, ns_ in enumerate(OTILE_SUPERS):
                ot = osb_pool.tile([128, ns_ * B_CORE], f16, tag="osb",
                                   name=f"osb{C}")
                for t2 in range(ns_):
                    t = t0 + t2
                    ps = ps_pool.tile([128, B_CORE], f32, tag="ps")
                    for u in range(2):
                        k = 2 * t + u
                        slab, off = slab_of_pair[k]
                        # column-tiling: the two pair matmuls land on
                        # disjoint PE column groups and stream concurrently
                        kw = dict(tile_position=(0, 64 * u)) if TILEPOS else {}
                        nc.tensor.matmul(
                            ps[64 * u:64 * u + 64, :],
                            w_sb[:, k * 64:(k + 1) * 64],
                            slab[:, off:off + B_CORE],
                            start=True, stop=True, **kw)
                    # bias-add evacuation, fp16 cast on write.  all-DVE
                    # keeps InstActivation out of the program (no ACT table
                    # load in the preamble) and the scalar engine free to
                    # issue stores promptly
                    dst = ot[:, t2 * B_CORE:(t2 + 1) * B_CORE]
                    if EVAC == "mix" and t2 % 2 == 0:
                        nc.scalar.add(dst, ps[:], bias_sb[:, t:t + 1])
                    else:
                        nc.vector.tensor_scalar_add(dst, ps[:],
                                                    bias_sb[:, t:t + 1])
                oeng = nc.sync if C in SYNC_OTILES else nc.scalar
                oeng.dma_start(
                    y_ap[:, t0 * B_CORE:(t0 + ns_) * B_CORE], ot[:])
                t0 += ns_

    nc.compile()
    _COMPILED["nc"] = nc
    return nc


def _prep_in_maps(x, weight, bias):
    x = np.asarray(x, dtype=np.float32)
    weight = np.asarray(weight, dtype=np.float32)
    bias = np.asarray(bias, dtype=np.float32)

    # x -> per-core PE-ready transposed fp16: xt[c][64h+s, 512k+b]
    #    = x[512c+b, 64(2k+h)+s]
    xt = x.reshape(N_CORES, B_CORE, N_PAIR, 2, STEP)    # [c, b, k, h, s]
    xt = xt.transpose(0, 3, 4, 2, 1)                    # [c, h, s, k, b]
    xt = np.ascontiguousarray(xt, dtype=np.float16)
    xt = xt.reshape(N_CORES, 128, N_PAIR * B_CORE)

    # gathered weight slice: Wg[j, s] = weight[j, (j%128)*64 + s]
    j = np.arange(OUT_F)
    Wg = weight.reshape(OUT_F, GROUPS, STEP)[j, j % GROUPS]   # [4096, 64]
    Wk = Wg.reshape(M_PER_G, N_PAIR, 2, STEP)                 # [m, k, h, s]
    # compact stationary halves: wc[64h+s, 32k+m] = Wk[m, k, h, s]
    wc = np.empty((2, STEP, N_PAIR, M_PER_G), dtype=np.float32)
    for h in range(2):
        wc[h] = Wk[:, :, h, :].transpose(2, 1, 0)             # [s, k, m]
    wc = wc.reshape(128, N_PAIR * M_PER_G)

    # bias in transposed-output layout: bias_p[64u+32h+m, t]
    #    = bias[m*128 + 4t + 2u + h]
    b4 = bias.reshape(M_PER_G, N_SUPER, 2, 2)                 # [m, t, u, h]
    bias_p = b4.transpose(2, 3, 0, 1).reshape(128, N_SUPER)

    w_c = np.ascontiguousarray(
        np.concatenate([wc, bias_p], axis=1), dtype=np.float16)

    in_maps = []
    for c in range(N_CORES):
        in_maps.append({
            "x_s": xt[c],
            "w_c": w_c,
        })
    return in_maps


def _unscramble(results):
    # y[64u+32h+m, 512t+b] = out[512c+b, m*128 + 4t + 2u + h]
    out = np.empty((BATCH, OUT_F), dtype=np.float32)
    for c in range(N_CORES):
        y = np.asarray(results[c]["out_s"])                  # [128, 16384] f16
        o = y.reshape(2, 2, M_PER_G, N_SUPER, B_CORE)        # [u, h, m, t, b]
        o = o.transpose(4, 2, 3, 0, 1)                       # [b, m, t, u, h]
        out[c * B_CORE:(c + 1) * B_CORE] = o.reshape(
            B_CORE, OUT_F).astype(np.float32)
    return out


def kernel(x, weight, bias):
    nc = _build()
    in_maps = _prep_in_maps(x, weight, bias)
    res = run_bass_kernel_spmd(nc, in_maps, core_ids=list(range(N_CORES)))
    return _unscramble(res.results)
